# revision 6
# baseline (speedup 1.0000x reference)
"""Hamming-distance embedding kernel for Trainium2 (8 NeuronCores, SPMD).

Math: for binary x in {0,1}^(B,L), refs in {0,1}^(D,L):
    hamming[b,d]   = sum_x[b] + sum_r[d] - 2*dot[b,d]
    out            = (hamming - L/2) / (0.5*sqrt(L))
Substituting a = 2x-1, c = 1-2r (both in {-1,+1}):
    hamming - L/2  = 0.5 * sum_l a[b,l]*c[d,l]
    out[b,d]       = (a @ c^T)[b,d] / sqrt(L)
So the whole module is ONE {+-1} matmul with a scalar scale. The +-1 values
are exact in bf16 and the f32 PSUM accumulation of +-1 products is exact.

Sharding: data-parallel over batch; refs replicated. Host side transposes
both operands (pure relayout, dtypes preserved) so the contraction dim L is
the SBUF partition dim for both matmul operands with contiguous DMA rows.
Each DMA chunk packs TWO consecutive l-rows per SBUF partition (a pure
reshape of the transposed array), doubling the per-descriptor contiguous
run (4 KiB x / 8 KiB refs) and halving DMA instruction count. The l-order
of a and c tiles match, so the contraction is unaffected.

Raw bass (no TileContext): this container's walrus rejects instructions
with more than a couple of attached sync waits, which Tile's kernel-tail
drain always exceeds. Raw engine blocks with one explicit wait_ge per
dependency stay within the budget. Per-chunk DMA semaphores (threshold 16)
make waits order-independent across HWDGE queues.

Pipeline per core:
  SP   : issue the 8 refs-chunk DMAs up front; drain output DMAs at tail
  ACT  : issue the 8 x-chunk DMAs, then a_bf[c] = bf16(2*x - 1)
  DVE  : c_bf[c] = bf16(1 - 2*r); out_sb[bi] = psum[bi] * 1/sqrt(L)
  PE   : psum[bi][dh] += a_bf[c][:,ls,bi].T @ c_bf[c][:,ls,dh] (16 steps)
"""

import math
from contextlib import ExitStack

import numpy as np

import concourse.bass as bass
import concourse.mybir as mybir
from concourse.bass_utils import run_bass_kernel_spmd

N_CORES = 8
B, D, L = 4096, 1024, 2048
B_SHARD = B // N_CORES  # 512

P = 128          # SBUF partitions / matmul contraction tile
N_TILE = 512     # matmul free-dim tile (one PSUM bank of f32)
RPP = 2          # l-rows packed per partition per DMA chunk


def build_nc(b_shard: int = B_SHARD, d: int = D, l_dim: int = L) -> bass.Bass:
    chunks = l_dim // (P * RPP)    # 8 DMA chunks; 2 matmul l-steps each
    b_chunks = b_shard // P
    d_halves = d // N_TILE
    scale = 1.0 / math.sqrt(l_dim)

    nc = bass.Bass()
    # (chunks, P, RPP*b_shard) is a pure reshape of row-major (l_dim, b_shard)
    xT = nc.declare_dram_parameter(
        "xT", [chunks, P, RPP * b_shard], mybir.dt.int32, isOutput=False)
    refsT = nc.declare_dram_parameter(
        "refsT", [chunks, P, RPP * d], mybir.dt.float32, isOutput=False)
    out = nc.declare_dram_parameter(
        "out", [b_shard, d], mybir.dt.float32, isOutput=True)

    with ExitStack() as ctx:
        xt_raw = [ctx.enter_context(
            nc.sbuf_tensor(f"xt{i}", [P, RPP * b_shard], mybir.dt.int32))
            for i in range(chunks)]
        rt_raw = [ctx.enter_context(
            nc.sbuf_tensor(f"rt{i}", [P, RPP * d], mybir.dt.float32))
            for i in range(chunks)]
        a_bf = [ctx.enter_context(
            nc.sbuf_tensor(f"ab{i}", [P, RPP * b_shard], mybir.dt.bfloat16))
            for i in range(chunks)]
        c_bf = [ctx.enter_context(
            nc.sbuf_tensor(f"cb{i}", [P, RPP * d], mybir.dt.bfloat16))
            for i in range(chunks)]
        out_sb = [ctx.enter_context(
            nc.sbuf_tensor(f"os{i}", [P, d], mybir.dt.float32))
            for i in range(b_chunks)]
        psum = [[ctx.enter_context(
            nc.psum_tensor(f"pm{bi}_{dh}", [P, N_TILE], mybir.dt.float32))
            for dh in range(d_halves)] for bi in range(b_chunks)]

        sem_x = [ctx.enter_context(nc.semaphore(f"sx{i}")) for i in range(chunks)]
        sem_r = [ctx.enter_context(nc.semaphore(f"sr{i}")) for i in range(chunks)]
        sem_a = ctx.enter_context(nc.semaphore("sa"))
        sem_c = ctx.enter_context(nc.semaphore("sc"))
        sem_mm = ctx.enter_context(nc.semaphore("smm"))
        sem_cp = ctx.enter_context(nc.semaphore("scp"))
        sem_out = ctx.enter_context(nc.semaphore("so"))

        with nc.Block() as block:

            @block.sync
            def _(sync):
                for c in range(chunks):
                    sync.dma_start(out=rt_raw[c][:], in_=refsT[c]
                                   ).then_inc(sem_r[c], 16)
                for bi in range(b_chunks):
                    sync.wait_ge(sem_cp, bi + 1)
                    sync.dma_start(out=out[bi * P:(bi + 1) * P, :], in_=out_sb[bi][:]
                                   ).then_inc(sem_out, 16)
                sync.wait_ge(sem_out, 16 * b_chunks)

            @block.scalar
            def _(scalar):
                # x loads go on ACT's HWDGE queues, in parallel with SP's
                # refs loads; ACT then consumes them for the affine cast.
                for c in range(chunks):
                    scalar.dma_start(out=xt_raw[c][:], in_=xT[c]
                                     ).then_inc(sem_x[c], 16)
                for c in range(chunks):
                    scalar.wait_ge(sem_x[c], 16)
                    nc.scalar.activation(
                        a_bf[c][:], xt_raw[c][:],
                        mybir.ActivationFunctionType.Copy, bias=-1.0, scale=2.0,
                    ).then_inc(sem_a, 1)

            @block.vector
            def _(vector):
                for c in range(chunks):
                    vector.wait_ge(sem_r[c], 16)
                    nc.vector.tensor_scalar(
                        out=c_bf[c][:], in0=rt_raw[c][:],
                        scalar1=-2.0, scalar2=1.0,
                        op0=mybir.AluOpType.mult, op1=mybir.AluOpType.add,
                    ).then_inc(sem_c, 1)
                for bi in range(b_chunks):
                    vector.wait_ge(sem_mm, d_halves * (bi + 1))
                    for dh in range(d_halves):
                        ins = nc.vector.tensor_scalar_mul(
                            out_sb[bi][:, dh * N_TILE:(dh + 1) * N_TILE],
                            psum[bi][dh][:], scale)
                    ins.then_inc(sem_cp, 1)

            @block.tensor
            def _(tensor):
                last = (chunks - 1, RPP - 1)
                for c in range(chunks):
                    tensor.wait_ge(sem_a, c + 1)
                    tensor.wait_ge(sem_c, c + 1)
                    for ls in range(RPP):
                        for bi in range(b_chunks):
                            for dh in range(d_halves):
                                mm = nc.tensor.matmul(
                                    psum[bi][dh][:],
                                    lhsT=a_bf[c][:, ls * b_shard + bi * P:
                                                 ls * b_shard + (bi + 1) * P],
                                    rhs=c_bf[c][:, ls * d + dh * N_TILE:
                                                ls * d + (dh + 1) * N_TILE],
                                    start=(c == 0 and ls == 0),
                                    stop=((c, ls) == last),
                                )
                                if (c, ls) == last:
                                    mm.then_inc(sem_mm, 1)

    return nc


_NC_CACHE: dict = {}


def kernel(x: np.ndarray, references: np.ndarray) -> np.ndarray:
    assert x.shape == (B, L) and references.shape == (D, L)
    chunks = L // (P * RPP)
    xT = np.ascontiguousarray(x.T)                    # (L, B) int32
    refsT = np.ascontiguousarray(references.T)        # (L, D) float32
    refs_feed = refsT.reshape(chunks, P, RPP * D)     # pure reshape

    in_maps = [
        {
            "xT": np.ascontiguousarray(
                xT[:, c * B_SHARD:(c + 1) * B_SHARD]
            ).reshape(chunks, P, RPP * B_SHARD),
            "refsT": refs_feed,
        }
        for c in range(N_CORES)
    ]

    if "nc" not in _NC_CACHE:
        _NC_CACHE["nc"] = build_nc()
    nc = _NC_CACHE["nc"]

    res = run_bass_kernel_spmd(nc, in_maps, core_ids=list(range(N_CORES)))
    outs = [res.results[c]["out"] for c in range(N_CORES)]
    return np.ascontiguousarray(np.concatenate(outs, axis=0), dtype=np.float32)


# revision 7
# speedup vs baseline: 1.1112x; 1.1112x over previous
"""Hamming-distance embedding kernel for Trainium2 (8 NeuronCores, SPMD).

Math: for binary x in {0,1}^(B,L), refs in {0,1}^(D,L):
    hamming[b,d]   = sum_x[b] + sum_r[d] - 2*dot[b,d]
    out            = (hamming - L/2) / (0.5*sqrt(L))
Substituting a = 2x-1, c = 1-2r (both in {-1,+1}):
    hamming - L/2  = 0.5 * sum_l a[b,l]*c[d,l]
    out[b,d]       = (a @ c^T)[b,d] / sqrt(L)
So the whole module is ONE {+-1} matmul with a scalar scale. The +-1 values
are exact in bf16 and the f32 PSUM accumulation of +-1 products is exact.

Sharding: data-parallel over batch; refs replicated. Host side transposes
both operands (pure relayout, dtypes preserved) so the contraction dim L is
the SBUF partition dim for both matmul operands with contiguous DMA rows.
Each DMA chunk packs TWO consecutive l-rows per SBUF partition (a pure
reshape of the transposed array), doubling the per-descriptor contiguous
run (4 KiB x / 8 KiB refs) and halving DMA instruction count. The l-order
of a and c tiles match, so the contraction is unaffected.

Raw bass (no TileContext): this container's walrus rejects instructions
with more than a couple of attached sync waits, which Tile's kernel-tail
drain always exceeds. Raw engine blocks with one explicit wait_ge per
dependency stay within the budget. Per-chunk DMA semaphores (threshold 16)
make waits order-independent across HWDGE queues.

Pipeline per core:
  SP   : issue the 8 refs-chunk DMAs up front; drain output DMAs at tail
  ACT  : issue the 8 x-chunk DMAs, then a_bf[c] = bf16(2*x - 1)
  DVE  : c_bf[c] = bf16(1 - 2*r); out_sb[bi] = psum[bi] * 1/sqrt(L)
  PE   : psum[bi][dh] += a_bf[c][:,ls,bi].T @ c_bf[c][:,ls,dh] (16 steps)
"""

import math
from contextlib import ExitStack

import numpy as np

import concourse.bass as bass
import concourse.mybir as mybir
from concourse.bass_utils import run_bass_kernel_spmd

N_CORES = 8
B, D, L = 4096, 1024, 2048
B_SHARD = B // N_CORES  # 512

P = 128          # SBUF partitions / matmul contraction tile
N_TILE = 512     # matmul free-dim tile (one PSUM bank of f32)
RPP = 2          # l-rows packed per partition per DMA chunk


def build_nc(b_shard: int = B_SHARD, d: int = D, l_dim: int = L) -> bass.Bass:
    chunks = l_dim // (P * RPP)    # 8 DMA chunks; 2 matmul l-steps each
    b_chunks = b_shard // P
    d_halves = d // N_TILE
    scale = 1.0 / math.sqrt(l_dim)

    nc = bass.Bass()
    # (chunks, P, RPP*b_shard) is a pure reshape of row-major (l_dim, b_shard)
    xT = nc.declare_dram_parameter(
        "xT", [chunks, P, RPP * b_shard], mybir.dt.int32, isOutput=False)
    refsT = nc.declare_dram_parameter(
        "refsT", [chunks, P, RPP * d], mybir.dt.float32, isOutput=False)
    out = nc.declare_dram_parameter(
        "out", [b_shard, d], mybir.dt.float32, isOutput=True)

    with ExitStack() as ctx:
        xt_raw = [ctx.enter_context(
            nc.sbuf_tensor(f"xt{i}", [P, RPP * b_shard], mybir.dt.int32))
            for i in range(chunks)]
        rt_raw = [ctx.enter_context(
            nc.sbuf_tensor(f"rt{i}", [P, RPP * d], mybir.dt.float32))
            for i in range(chunks)]
        a_bf = [ctx.enter_context(
            nc.sbuf_tensor(f"ab{i}", [P, RPP * b_shard], mybir.dt.bfloat16))
            for i in range(chunks)]
        c_bf = [ctx.enter_context(
            nc.sbuf_tensor(f"cb{i}", [P, RPP * d], mybir.dt.bfloat16))
            for i in range(chunks)]
        out_sb = [ctx.enter_context(
            nc.sbuf_tensor(f"os{i}", [P, d], mybir.dt.float32))
            for i in range(b_chunks)]
        psum = [[ctx.enter_context(
            nc.psum_tensor(f"pm{bi}_{dh}", [P, N_TILE], mybir.dt.float32))
            for dh in range(d_halves)] for bi in range(b_chunks)]

        sem_x = [ctx.enter_context(nc.semaphore(f"sx{i}")) for i in range(chunks)]
        sem_r = [ctx.enter_context(nc.semaphore(f"sr{i}")) for i in range(chunks)]
        sem_a = ctx.enter_context(nc.semaphore("sa"))
        sem_c = ctx.enter_context(nc.semaphore("sc"))
        sem_mm = ctx.enter_context(nc.semaphore("smm"))
        sem_cp = ctx.enter_context(nc.semaphore("scp"))
        sem_out = ctx.enter_context(nc.semaphore("so"))

        with nc.Block() as block:

            @block.sync
            def _(sync):
                # All loads on one queue, r/x interleaved so the queue serves
                # both streams proportionally (a split across SP+ACT queues
                # measured slower: imbalance + late second-queue bring-up).
                for c in range(chunks):
                    sync.dma_start(out=rt_raw[c][:], in_=refsT[c]
                                   ).then_inc(sem_r[c], 16)
                    sync.dma_start(out=xt_raw[c][:], in_=xT[c]
                                   ).then_inc(sem_x[c], 16)
                for bi in range(b_chunks):
                    sync.wait_ge(sem_cp, bi + 1)
                    sync.dma_start(out=out[bi * P:(bi + 1) * P, :], in_=out_sb[bi][:]
                                   ).then_inc(sem_out, 16)
                sync.wait_ge(sem_out, 16 * b_chunks)

            @block.scalar
            def _(scalar):
                for c in range(chunks):
                    scalar.wait_ge(sem_x[c], 16)
                    nc.scalar.activation(
                        a_bf[c][:], xt_raw[c][:],
                        mybir.ActivationFunctionType.Copy, bias=-1.0, scale=2.0,
                    ).then_inc(sem_a, 1)

            @block.vector
            def _(vector):
                for c in range(chunks):
                    vector.wait_ge(sem_r[c], 16)
                    nc.vector.tensor_scalar(
                        out=c_bf[c][:], in0=rt_raw[c][:],
                        scalar1=-2.0, scalar2=1.0,
                        op0=mybir.AluOpType.mult, op1=mybir.AluOpType.add,
                    ).then_inc(sem_c, 1)
                for bi in range(b_chunks):
                    vector.wait_ge(sem_mm, d_halves * (bi + 1))
                    for dh in range(d_halves):
                        ins = nc.vector.tensor_scalar_mul(
                            out_sb[bi][:, dh * N_TILE:(dh + 1) * N_TILE],
                            psum[bi][dh][:], scale)
                    ins.then_inc(sem_cp, 1)

            @block.tensor
            def _(tensor):
                last = (chunks - 1, RPP - 1)
                for c in range(chunks):
                    tensor.wait_ge(sem_a, c + 1)
                    tensor.wait_ge(sem_c, c + 1)
                    for ls in range(RPP):
                        for bi in range(b_chunks):
                            for dh in range(d_halves):
                                mm = nc.tensor.matmul(
                                    psum[bi][dh][:],
                                    lhsT=a_bf[c][:, ls * b_shard + bi * P:
                                                 ls * b_shard + (bi + 1) * P],
                                    rhs=c_bf[c][:, ls * d + dh * N_TILE:
                                                ls * d + (dh + 1) * N_TILE],
                                    start=(c == 0 and ls == 0),
                                    stop=((c, ls) == last),
                                )
                                if (c, ls) == last:
                                    mm.then_inc(sem_mm, 1)

    return nc


_NC_CACHE: dict = {}


def kernel(x: np.ndarray, references: np.ndarray) -> np.ndarray:
    assert x.shape == (B, L) and references.shape == (D, L)
    chunks = L // (P * RPP)
    xT = np.ascontiguousarray(x.T)                    # (L, B) int32
    refsT = np.ascontiguousarray(references.T)        # (L, D) float32
    refs_feed = refsT.reshape(chunks, P, RPP * D)     # pure reshape

    in_maps = [
        {
            "xT": np.ascontiguousarray(
                xT[:, c * B_SHARD:(c + 1) * B_SHARD]
            ).reshape(chunks, P, RPP * B_SHARD),
            "refsT": refs_feed,
        }
        for c in range(N_CORES)
    ]

    if "nc" not in _NC_CACHE:
        _NC_CACHE["nc"] = build_nc()
    nc = _NC_CACHE["nc"]

    res = run_bass_kernel_spmd(nc, in_maps, core_ids=list(range(N_CORES)))
    outs = [res.results[c]["out"] for c in range(N_CORES)]
    return np.ascontiguousarray(np.concatenate(outs, axis=0), dtype=np.float32)
